# revision 1
# baseline (speedup 1.0000x reference)
"""Trainium2 Bass kernel for the DialogueGNN gated multimodal fusion layer.

Computes, for N = B*L nodes (node n = b*L + t, batch-major flatten):
    ha = tanh(na @ Wa.T + ba)   (same for hv, hl)
    z_xy = sigmoid([nx, ny, nx*ny] @ Wxy.T + bxy)    for xy in {av, al, vl}
    h_xy = z_xy * hx + (1 - z_xy) * hy
    out  = concat([h_av, h_al, h_vl], axis=-1)       # (N, 3D) fp32

Strategy (8 NeuronCores, data-parallel over nodes):
  * Host: shard batches 16-per-core, pre-transpose activations to
    feature-major [2, 128, 16384] and cast to fp16 (halves input HBM
    traffic; fp16 keeps ~1e-3 accuracy vs fp32 reference).
  * Device: per 1024-node chunk
      - DMA feature-major fp16 activations,
      - GPSIMD elementwise products na*nv etc. (feature-major, fp16),
      - PE: activations are the *stationary* matmul operand; the [128,256]
        fp16 weight tile streams as rhs, plus a 3-column gate-weight rhs
        that reuses the loaded stationary (gate dots nearly free),
      - ACT: tanh/sigmoid drains of PSUM,
      - DVE: d = hx - hy (fp16 2x), then one fused scalar_tensor_tensor
        h = z*(hx-hy) + hy per 128-node tile (fp16 out),
      - DMA out [128, 8, 768] fp16 -> node-major rows; host upcasts to fp32.
"""

import os
import sys
from contextlib import ExitStack

import numpy as np

for _p in ("/opt/trn_rl_repo", "/root/.axon_site/_ro/trn_rl_repo"):
    if os.path.isdir(_p) and _p not in sys.path:
        sys.path.insert(0, _p)

import concourse.bass as bass
import concourse.bacc as bacc
import concourse.tile as tile
from concourse import mybir
from concourse.bass_utils import run_bass_kernel_spmd

L, B, D = 1024, 128, 256
N_CORES = 8
B_CORE = B // N_CORES          # 16 batches per core
N_CORE = B_CORE * L            # 16384 nodes per core
CHUNK = 1024                   # nodes per chunk
NTILE = CHUNK // 128           # 8 node-tiles of 128 per chunk
NCHUNK = N_CORE // CHUNK       # 16 chunks per core

MM_DT = mybir.dt.float16       # matmul / elementwise-intermediate dtype
NP_MM_DT = np.float16

F32 = mybir.dt.float32
AX = mybir.AluOpType


def _build_nc(with_bias: bool):
    """Build the Bass program (identical on all 8 cores)."""
    nc = bacc.Bacc("TRN2", target_bir_lowering=False, debug=False)

    xa = nc.dram_tensor("a_t", [2, 128, N_CORE], MM_DT, kind="ExternalInput")
    xv = nc.dram_tensor("v_t", [2, 128, N_CORE], MM_DT, kind="ExternalInput")
    xl = nc.dram_tensor("l_t", [2, 128, N_CORE], MM_DT, kind="ExternalInput")
    wm = nc.dram_tensor("w_main", [3, 2, 128, D], MM_DT, kind="ExternalInput")
    wg = nc.dram_tensor("w_gate", [6, 2, 128, 3], MM_DT, kind="ExternalInput")
    if with_bias:
        mb = nc.dram_tensor("b_main", [1, 3, D], MM_DT, kind="ExternalInput")
        gb = nc.dram_tensor("b_gate", [1, 3], MM_DT, kind="ExternalInput")
    out = nc.dram_tensor("out", [N_CORE, 3 * D], MM_DT, kind="ExternalOutput")

    with tile.TileContext(nc) as tc, ExitStack() as ctx:
        # pools are per-tag rings: bufs = ring depth per tag.
        const = ctx.enter_context(tc.tile_pool(name="const", bufs=1))
        io_in = ctx.enter_context(tc.tile_pool(name="io_in", bufs=3))
        prod_p = ctx.enter_context(tc.tile_pool(name="prod", bufs=2))
        h_p = ctx.enter_context(tc.tile_pool(name="h", bufs=3))
        d_p = ctx.enter_context(tc.tile_pool(name="d", bufs=2))
        z_p = ctx.enter_context(tc.tile_pool(name="z", bufs=3))
        out_p = ctx.enter_context(tc.tile_pool(name="out", bufs=2))
        ps_main = ctx.enter_context(
            tc.tile_pool(name="ps_main", bufs=3, space="PSUM"))
        ps_z = ctx.enter_context(tc.tile_pool(name="ps_z", bufs=2, space="PSUM"))

        # ---- constants ----
        w_main_sb = const.tile([128, 6, D], MM_DT)       # (mod, kh) -> idx m*2+kh
        nc.sync.dma_start(out=w_main_sb,
                          in_=wm.rearrange("m k p c -> p (m k) c"))
        w_gate_sb = const.tile([128, 12, 3], MM_DT)      # (stream, kh) -> s*2+kh
        nc.sync.dma_start(out=w_gate_sb,
                          in_=wg.rearrange("s k p c -> p (s k) c"))
        if with_bias:
            ones_sb = const.tile([1, 128], MM_DT)
            nc.vector.memset(ones_sb, 1.0)
            mb_sb = const.tile([1, 3, D], MM_DT)
            nc.sync.dma_start(out=mb_sb, in_=mb)
            gb_sb = const.tile([1, 3], MM_DT)
            nc.sync.dma_start(out=gb_sb, in_=gb)

        def emit_load(ch):
            """DMA-in of the feature-major activations."""
            sl = slice(ch * CHUNK, (ch + 1) * CHUNK)
            na = io_in.tile([128, 2, CHUNK], MM_DT, tag="na")
            nv = io_in.tile([128, 2, CHUNK], MM_DT, tag="nv")
            nl = io_in.tile([128, 2, CHUNK], MM_DT, tag="nl")
            for t_sb, t_dr in ((na, xa), (nv, xv), (nl, xl)):
                nc.sync.dma_start(
                    out=t_sb,
                    in_=t_dr[:, :, sl].rearrange("k p n -> p k n"))
            return na, nv, nl

        def emit_product(na, nv, nl, g):
            """One pairwise product (gate bilinear term) on DVE.  GpSimd
            is ~4x slower per element AND contends with DVE for SBUF
            ports (measured), so it stays idle."""
            x, y = ((na, nv), (na, nl), (nv, nl))[g]
            pg = prod_p.tile([128, 2, CHUNK], MM_DT, tag=f"p{g}")
            nc.vector.tensor_mul(pg, x, y)
            return pg

        def emit_compute(ch, ins, prods):
            na, nv, nl = ins
            # matmuls: activations stationary, weights moving
            z_ps = ps_z.tile([128, 3 * NTILE], F32)
            hs = []
            for m, src in enumerate((na, nv, nl)):
                h_m = h_p.tile([128, NTILE * D], MM_DT, tag=f"h{m}")
                for half in range(2):
                    ps = ps_main.tile([128, 4 * D], F32, tag="hps")
                    for jj in range(4):
                        j = half * 4 + jj
                        for kh in range(2):
                            lhs = src[:, kh, j * 128:(j + 1) * 128]
                            nc.tensor.matmul(
                                ps[:, jj * D:(jj + 1) * D],
                                lhsT=lhs, rhs=w_main_sb[:, m * 2 + kh, :],
                                start=(kh == 0),
                                stop=(kh == 1 and not with_bias),
                            )
                            nc.tensor.matmul(
                                z_ps[:, j * 3:(j + 1) * 3],
                                lhsT=lhs, rhs=w_gate_sb[:, m * 2 + kh, :],
                                start=(m == 0 and kh == 0 and j == 0),
                                stop=False,
                                skip_group_check=True,
                            )
                        if with_bias:
                            nc.tensor.matmul(
                                ps[:, jj * D:(jj + 1) * D],
                                lhsT=ones_sb, rhs=mb_sb[:, m, :],
                                start=False, stop=True,
                            )
                    # tanh drain PSUM -> SBUF (fp16 out)
                    nc.scalar.activation(
                        out=h_m[:, half * 4 * D:(half + 1) * 4 * D], in_=ps,
                        func=mybir.ActivationFunctionType.Tanh)
                hs.append(h_m)

            # gate contributions from the products
            for g, pg in enumerate(prods):
                for j in range(NTILE):
                    for kh in range(2):
                        last = (g == 2 and kh == 1 and not with_bias)
                        nc.tensor.matmul(
                            z_ps[:, j * 3:(j + 1) * 3],
                            lhsT=pg[:, kh, j * 128:(j + 1) * 128],
                            rhs=w_gate_sb[:, (3 + g) * 2 + kh, :],
                            start=False, stop=last, skip_group_check=True,
                        )
            if with_bias:
                for j in range(NTILE):
                    nc.tensor.matmul(
                        z_ps[:, j * 3:(j + 1) * 3], lhsT=ones_sb, rhs=gb_sb,
                        start=False, stop=True, skip_group_check=True,
                    )

            z_sb = z_p.tile([128, 3 * NTILE], F32)
            nc.scalar.activation(out=z_sb, in_=z_ps,
                                 func=mybir.ActivationFunctionType.Sigmoid)
            return hs, z_sb

        def emit_blend(ch, hs, z_sb, next_ins):
            """Gated fusion h = z*(hx - hy) + hy, then DMA out.

            d = hx - hy chunk-wide on DVE (fp16 2x).  t = z * d runs as
            24 per-(j,gate) 256-col ops with a per-partition scalar z,
            split 12/12 between DVE tensor_scalar and ACT scaled-copy to
            balance the engines.  h = t + hy chunk-wide on DVE.

            The next chunk's three products are interleaved into this
            DVE program (p0 first, p1/p2 mid) so PE's product-gate
            matmuls and the sigmoid are never starved at period end.
            """
            sl = slice(ch * CHUNK, (ch + 1) * CHUNK)
            last = next_ins is None
            pairs = ((hs[0], hs[1]), (hs[0], hs[2]), (hs[1], hs[2]))
            h16 = out_p.tile([128, NTILE, 3 * D], MM_DT, tag="h16")
            tgs = []
            for o, (hx, hy) in enumerate(pairs):
                dg = d_p.tile([128, NTILE * D], MM_DT, tag=f"d{o}")
                nc.vector.tensor_sub(dg, hx, hy)
                tg = d_p.tile([128, NTILE * D], MM_DT, tag=f"t{o}")
                for j in range(NTILE):
                    zap = z_sb[:, j * 3 + o: j * 3 + o + 1]
                    if (j + o) % 2 == 0:
                        nc.scalar.activation(
                            out=tg[:, j * D:(j + 1) * D],
                            in_=dg[:, j * D:(j + 1) * D],
                            func=mybir.ActivationFunctionType.Copy,
                            scale=zap)
                    else:
                        nc.vector.tensor_scalar_mul(
                            tg[:, j * D:(j + 1) * D],
                            dg[:, j * D:(j + 1) * D],
                            zap)
                tgs.append(tg)
                if not last:
                    nc.vector.tensor_add(
                        h16[:, :, o * D:(o + 1) * D],
                        tg.rearrange("p (j d) -> p j d", d=D),
                        hy.rearrange("p (j d) -> p j d", d=D))
            if last:
                for j0, j1 in ((0, NTILE // 2), (NTILE // 2, NTILE)):
                    for o in range(3):
                        hy = pairs[o][1]
                        nc.vector.tensor_add(
                            h16[:, j0:j1, o * D:(o + 1) * D],
                            tgs[o].rearrange(
                                "p (j d) -> p j d", d=D)[:, j0:j1],
                            hy.rearrange(
                                "p (j d) -> p j d", d=D)[:, j0:j1])
                    nc.sync.dma_start(
                        out=out[sl, :].rearrange(
                            "(j p) c -> p j c", p=128)[:, j0:j1],
                        in_=h16[:, j0:j1])
            else:
                nc.sync.dma_start(
                    out=out[sl, :].rearrange("(j p) c -> p j c", p=128),
                    in_=h16)
            # next chunk's products at DVE period end: by now its input
            # DMA has long landed, and PE needs them only next period.
            return ([emit_product(*next_ins, g) for g in range(3)]
                    if next_ins is not None else None)

        # Software pipeline: emit blend of chunk ch-1 between the DMA-in
        # and the compute of chunk ch, so every instruction's cross-engine
        # dependencies are one full stage old (no intra-chunk ping-pong).
        pending = None
        for ch in range(NCHUNK):
            ins = emit_load(ch)
            if pending is not None:
                prods = emit_blend(*pending, ins)
            else:
                prods = [emit_product(*ins, g) for g in range(3)]
            hs, z_sb = emit_compute(ch, ins, prods)
            pending = (ch, hs, z_sb)
        emit_blend(*pending, None)

    nc.compile()
    return nc


_CACHE = {}


def _get_nc(with_bias: bool):
    key = ("nc", with_bias)
    if key not in _CACHE:
        _CACHE[key] = _build_nc(with_bias)
    return _CACHE[key]


def _prep_weights(Wa, Wv, Wl, Wav, Wal, Wvl):
    # w_main[m, kh] = W.T[kh*128:(kh+1)*128, :]  ([128, D] slice of [K, M])
    wm = np.stack([
        np.ascontiguousarray(W.T.reshape(2, 128, D))
        for W in (Wa, Wv, Wl)
    ]).astype(NP_MM_DT)                               # [3, 2, 128, D]
    # gate vectors, split into per-stream blocks of 3 columns
    wav, wal, wvl = Wav[0], Wal[0], Wvl[0]            # (768,)
    Z = np.zeros(D, np.float32)
    blocks = [
        (wav[0:D],      wal[0:D],      Z),            # stream na
        (wav[D:2 * D],  Z,             wvl[0:D]),     # stream nv
        (Z,             wal[D:2 * D],  wvl[D:2 * D]),  # stream nl
        (wav[2 * D:],   Z,             Z),            # stream na*nv
        (Z,             wal[2 * D:],   Z),            # stream na*nl
        (Z,             Z,             wvl[2 * D:]),  # stream nv*nl
    ]
    wg = np.stack([
        np.stack([np.asarray(c0), np.asarray(c1), np.asarray(c2)], axis=1)
        .reshape(2, 128, 3)
        for (c0, c1, c2) in blocks
    ]).astype(NP_MM_DT)                               # [6, 2, 128, 3]
    return wm, wg


def _prep_acts(x, c):
    """x: (L, B, D) fp32 -> core-c feature-major [2, 128, N_CORE] fp16."""
    xc = x[:, c * B_CORE:(c + 1) * B_CORE, :]         # (L, 16, D)
    xt = np.ascontiguousarray(xc.astype(NP_MM_DT).transpose(2, 1, 0))
    return xt.reshape(2, 128, N_CORE)                 # k-major, n = b*L + t


def kernel(**inputs) -> np.ndarray:
    a = np.asarray(inputs["a"], np.float32)
    v = np.asarray(inputs["v"], np.float32)
    l = np.asarray(inputs["l"], np.float32)
    names = ("Wa", "Wv", "Wl", "Wav", "Wal", "Wvl")
    Wa, Wv, Wl, Wav, Wal, Wvl = (np.asarray(inputs[n], np.float32)
                                 for n in names)
    biases = {n: np.asarray(inputs[n], np.float32)
              for n in ("ba", "bv", "bl", "bav", "bal", "bvl")}
    with_bias = any(np.any(b) for b in biases.values())

    nc = _get_nc(with_bias)
    wm, wg = _prep_weights(Wa, Wv, Wl, Wav, Wal, Wvl)

    in_maps = []
    for c in range(N_CORES):
        m = {
            "a_t": _prep_acts(a, c),
            "v_t": _prep_acts(v, c),
            "l_t": _prep_acts(l, c),
            "w_main": wm,
            "w_gate": wg,
        }
        if with_bias:
            m["b_main"] = np.stack(
                [biases["ba"], biases["bv"], biases["bl"]])[None].astype(NP_MM_DT)
            m["b_gate"] = np.array(
                [[biases["bav"][0], biases["bal"][0], biases["bvl"][0]]],
                NP_MM_DT)
        in_maps.append(m)

    trace = bool(int(os.environ.get("KERNEL_TRACE", "0")))
    kw = {}
    if trace and os.environ.get("KERNEL_TRACE_DIR"):
        kw["tmpdir"] = os.environ["KERNEL_TRACE_DIR"]
    res = run_bass_kernel_spmd(nc, in_maps, core_ids=list(range(N_CORES)),
                               trace=trace, **kw)
    _CACHE["last_results"] = res
    return np.concatenate(
        [res.results[c]["out"].astype(np.float32) for c in range(N_CORES)],
        axis=0)

